# revision 11
# baseline (speedup 1.0000x reference)
"""Trainium2 Bass kernel for nn_ContrastiveLearning (NT-Xent over pairwise
symmetrized-KL of diagonal Gaussians).

Math (equivalent reformulation of the reference):
  loss[i,j] = -0.25*(A[i,j] + A[j,i] + md[i,j] + md[j,i] - 2D)   (ld terms cancel)
  A[i,j]+A[j,i] + md[i,j]+md[j,i] - (q_i + q_j)
      = <u_i,s_j> + <s_i,u_j> - 2<w_i,m_j> - 2<m_i,w_j>  =: total[i,j]
  with u=1/sigma, s=sigma+mu^2, w=mu/sigma, m=mu, q_x=<mu^2,1/sigma>_x.
  Row-constant factors cancel in lf_i = -log2(num_i)+log2(den_i)+log2(cnt_i),
  so the device computes E[i,j] = exp(-0.25*(total[i,j] + q_j + pen_j) + C)
  where pen_j = BIG kills padding columns, the diagonal is killed by adding
  BIG*I to total via an extra matmul, and C keeps fp32 exp in range.
  num_i = sum_j E*[lab_i==lab_j],  den_i = sum_j E.
  cnt, log2 and the final masked mean are O(N) host work.

Sharding: anchor rows are data-parallel over 8 cores. Invalid tokens
(mask==0) are compacted out on the host, the remainder padded to a multiple
of 512. Each core receives the token axis rotated by core*N/8 so its anchors
are always tokens [0, n_anchor) — a single SPMD program with static
addressing.

Perf notes (learned from traces):
  - no gpsimd (slow, and it throttles other engines' SBUF access)
  - all matmul operands bf16 (2.4GHz PE streaming; errors average out over
    ~1M pairs, final rel err ~1e-5 vs 2e-2 budget)
  - host pre-transposes/pre-packs all arrays per-partition-contiguous and
    pre-casts to bf16, so every DMA moves multi-KB descriptors
  - phase-2 j-tiles are emitted one group behind phase-1 so the tensor queue
    always has independent work while a group's epilogue completes
  - vector reciprocal_approx_fast (18 bits) instead of reciprocal (5x)
  - single Exp activation table load (no Ln anywhere)
"""
import numpy as np

EMBED_DIM = 128
H_DIM = 768
C_SHIFT = 40.0           # global exponent shift, cancels between num and den
BIG = 1024.0 * 1024.0    # diag/pad kill: exp(-0.25*BIG) == 0 in fp32
SQRT_BIG = 1024.0

_cache = {}


def _bf16(x):
    import ml_dtypes
    return np.asarray(x, dtype=np.float32).astype(ml_dtypes.bfloat16)


def _build(n_pad):
    import concourse.bass as bass
    import concourse.tile as tile
    from concourse import bacc, mybir, masks

    f32 = mybir.dt.float32
    bf16 = mybir.dt.bfloat16
    AF = mybir.ActivationFunctionType
    ALU = mybir.AluOpType
    AX = mybir.AxisListType

    n_groups = n_pad // 512          # 512-token groups == J-tiles
    n_anchor = n_pad // 8            # anchors per core (multiple of 64)
    n_itiles = (n_anchor + 127) // 128
    assert n_anchor <= 512

    nc = bacc.Bacc(None, target_bir_lowering=False, debug=False)
    # emb slab kk: [128, n_pad] bf16, emb<kk>[p, t] = emb[token t, feature kk*128+p]
    emb_ds = [nc.declare_dram_parameter(f"emb{kk}", [128, n_pad], bf16, isOutput=False)
              for kk in range(6)]
    # wA: [128, 2*6*128] bf16: wmu then wsig, wA[p, (w*6+k)*128+c] = W[k*128+p, c]
    wA_d = nc.declare_dram_parameter("wA", [128, 2 * 6 * 128], bf16, isOutput=False)
    # small: [128, 3+n_itiles] f32: [bmu, bsig, -2*bmu, labr tiles...]
    small_d = nc.declare_dram_parameter("small", [128, 3 + n_itiles], f32, isOutput=False)
    labc_d = nc.declare_dram_parameter("labc", [n_pad], bf16, isOutput=False)
    pen_d = nc.declare_dram_parameter("pen", [n_pad], bf16, isOutput=False)
    out_d = nc.declare_dram_parameter("out", [128, n_itiles * 2], f32, isOutput=True)

    with tile.TileContext(nc) as tc:
        with tc.tile_pool(name="const", bufs=1) as const, \
             tc.tile_pool(name="feat", bufs=1) as feat, \
             tc.tile_pool(name="work", bufs=2) as work, \
             tc.tile_pool(name="acc", bufs=1) as accp, \
             tc.tile_pool(name="psB", bufs=2, space="PSUM") as psB, \
             tc.tile_pool(name="psC", bufs=2, space="PSUM") as psC:

            # ---------------- bulk loads (small first, emb slabs parallel) --
            small_t = const.tile([128, 3 + n_itiles], f32)
            nc.sync.dma_start(small_t[:], small_d[:])

            w_all = const.tile([128, 2, 6, 128], bf16)
            nc.sync.dma_start(w_all[:], wA_d[:])
            emb_ks = [const.tile([128, n_pad], bf16, tag=f"emb{kk}", name=f"emb{kk}")
                      for kk in range(6)]
            h_ks = [const.tile([128, n_pad], bf16, tag=f"h{kk}", name=f"h{kk}")
                    for kk in range(6)]
            gs0 = slice(0, 512)
            for kk in range(6):
                nc.sync.dma_start(emb_ks[kk][:, gs0], emb_ds[kk][:, gs0])
            # qp row 1 = pen (row 0 = q, written per group)
            qp = feat.tile([2, n_pad], bf16)
            nc.sync.dma_start(qp[1:2, :], pen_d.rearrange("(o n) -> o n", o=1))
            labcb_t = const.tile([128, n_pad], bf16)
            nc.sync.dma_start(labcb_t[:], labc_d[:].partition_broadcast(128))
            for g in range(1, n_groups):
                gsl = slice(g * 512, (g + 1) * 512)
                for kk in range(6):
                    nc.sync.dma_start(emb_ks[kk][:, gsl], emb_ds[kk][:, gsl])

            # ---------------- small consts ----------------
            identity0 = const.tile([128, 128], f32)
            masks.make_identity(nc, identity0[:])
            eqd_b = const.tile([128, 128], bf16)
            nc.vector.tensor_scalar(eqd_b[:], identity0[:], SQRT_BIG, None, ALU.mult)
            onescol_b = const.tile([128, 1], bf16)
            nc.vector.memset(onescol_b[:], 1.0)
            ones2_b = const.tile([2, 128], bf16)
            nc.vector.memset(ones2_b[:], 1.0)
            cbias_t = const.tile([128, 1], f32)
            nc.vector.memset(cbias_t[:], C_SHIFT)

            # ---------------- persistent feature tensors (bf16) -----------
            u_f = feat.tile([128, n_pad], bf16)      # 1/sigma          (G_col)
            s_f = feat.tile([128, n_pad], bf16)      # sigma + mu^2     (G_col)
            m2_f = feat.tile([128, n_pad], bf16)     # -2*mu            (G_col)
            w2_f = feat.tile([128, n_pad], bf16)     # -2*mu/sigma      (G_col)
            mu_a = feat.tile([128, n_anchor], bf16)  # mu       (G_row anchors)
            w_a = feat.tile([128, n_anchor], bf16)   # mu/sigma (G_row anchors)

            num_sl = accp.tile([128, n_itiles * n_groups], f32)
            den_sl = accp.tile([128, n_itiles * n_groups], f32)

            def phase2(t, jt):
                m = min(128, n_anchor - t * 128)
                isl = slice(t * 128, t * 128 + m)
                jsl = slice(jt * 512, (jt + 1) * 512)
                doff = t * 128                   # diagonal lives in j-tile 0
                idx = t * n_groups + jt
                ps2 = psC.tile([128, 512], f32, tag="p2")
                nc.tensor.matmul(ps2[0:m, :], s_f[:, isl], u_f[:, jsl],
                                 start=True, stop=False)
                nc.tensor.matmul(ps2[0:m, :], u_f[:, isl], s_f[:, jsl],
                                 start=False, stop=False)
                nc.tensor.matmul(ps2[0:m, :], w_a[:, isl], m2_f[:, jsl],
                                 start=False, stop=False)
                nc.tensor.matmul(ps2[0:m, :], mu_a[:, isl], w2_f[:, jsl],
                                 start=False, stop=False)
                last = (jt != 0)
                nc.tensor.matmul(ps2[0:m, :], ones2_b[:, 0:m], qp[:, jsl],
                                 start=False, stop=last)
                if jt == 0:
                    nc.tensor.matmul(ps2[0:m, doff:doff + 128],
                                     eqd_b[:, 0:m], eqd_b[:],
                                     start=False, stop=True)
                e_t = work.tile([128, 512], bf16, tag="E")
                nc.scalar.activation(e_t[0:m, :], ps2[0:m, :], AF.Exp,
                                     scale=-0.25, bias=cbias_t[0:m, :],
                                     accum_out=den_sl[0:m, idx:idx + 1])
                labq = work.tile([128, 512], bf16, tag="labq")
                nc.vector.tensor_scalar(labq[0:m, :], labcb_t[0:m, jsl],
                                        small_t[0:m, 3 + t:4 + t], None, ALU.is_equal)
                msk = work.tile([128, 512], bf16, tag="msk")
                nc.vector.scalar_tensor_tensor(msk[0:m, :], e_t[0:m, :], 1.0,
                                               labq[0:m, :], ALU.mult, ALU.mult,
                                               accum_out=num_sl[0:m, idx:idx + 1])

            # ---------------- phase 1 (+ phase 2 one group behind) --------
            for g in range(n_groups):
                gs = slice(g * 512, (g + 1) * 512)
                for kk in range(6):
                    if kk % 2 == 0:
                        nc.scalar.activation(h_ks[kk][:, gs], emb_ks[kk][:, gs], AF.Relu)
                    else:
                        nc.vector.tensor_scalar_max(h_ks[kk][:, gs], emb_ks[kk][:, gs], 0.0)
                ps_mu = psB.tile([128, 512], f32, tag="mu")
                ps_z = psB.tile([128, 512], f32, tag="z")
                for kk in range(6):
                    nc.tensor.matmul(ps_mu[:], w_all[:, 0, kk, :],
                                     h_ks[kk][:, gs],
                                     start=(kk == 0), stop=(kk == 5))
                for kk in range(6):
                    nc.tensor.matmul(ps_z[:], w_all[:, 1, kk, :],
                                     h_ks[kk][:, gs],
                                     start=(kk == 0), stop=(kk == 5))

                # mu-path epilogue
                psq = work.tile([128, 512], f32, tag="psq")
                nc.scalar.activation(psq[:], ps_mu[:], AF.Square, bias=small_t[:, 0:1])
                nc.vector.tensor_scalar(m2_f[:, gs], ps_mu[:], small_t[:, 0:1],
                                        -2.0, ALU.add, ALU.mult)
                # sigma-path epilogue: sigma = exp(min(z,0)) + 1e-14 + relu(z)
                zm = work.tile([128, 512], f32, tag="zm")
                nc.vector.tensor_scalar(zm[:], ps_z[:], small_t[:, 1:2], 0.0,
                                        ALU.add, ALU.min)
                zp = work.tile([128, 512], f32, tag="zp")
                nc.scalar.activation(zp[:], ps_z[:], AF.Relu, bias=small_t[:, 1:2])
                e1 = work.tile([128, 512], f32, tag="e1")
                nc.scalar.activation(e1[:], zm[:], AF.Exp)
                sig_g = work.tile([128, 512], f32, tag="sig_g")
                nc.vector.scalar_tensor_tensor(sig_g[:], e1[:], 1e-14, zp[:],
                                               ALU.add, ALU.add)
                u_s = work.tile([128, 512], f32, tag="u_s")
                nc.vector.reciprocal_approx_fast(u_s[:], sig_g[:])
                nc.scalar.copy(u_f[:, gs], u_s[:])
                nc.vector.tensor_add(s_f[:, gs], psq[:], sig_g[:])
                nc.vector.tensor_mul(w2_f[:, gs], m2_f[:, gs], u_f[:, gs])
                pu_g = work.tile([128, 512], bf16, tag="pu_g")
                nc.vector.tensor_mul(pu_g[:], psq[:], u_s[:])
                if g == 0:
                    nc.vector.tensor_scalar_mul(mu_a[:], m2_f[:, 0:n_anchor], -0.5)
                    nc.vector.tensor_scalar_mul(w_a[:], w2_f[:, 0:n_anchor], -0.5)
                # q row: ones^T @ pu
                ps_q = psC.tile([1, 512], f32, tag="q")
                nc.tensor.matmul(ps_q[:], onescol_b[:], pu_g[:],
                                 start=True, stop=True)
                nc.scalar.copy(qp[0:1, gs], ps_q[:])

                # phase-2 column tiles, one group behind
                if g >= 1:
                    for t in range(n_itiles):
                        phase2(t, g - 1)
            for t in range(n_itiles):
                phase2(t, n_groups - 1)

            # ---------------- final row reductions + single store ---------
            nd = accp.tile([128, n_itiles * 2], f32)
            for t in range(n_itiles):
                sl = slice(t * n_groups, (t + 1) * n_groups)
                nc.vector.tensor_reduce(nd[:, 2 * t:2 * t + 1], num_sl[:, sl],
                                        AX.X, ALU.add)
                nc.vector.tensor_reduce(nd[:, 2 * t + 1:2 * t + 2], den_sl[:, sl],
                                        AX.X, ALU.add)
            nc.sync.dma_start(out_d[:], nd[:])

    nc.compile()
    return nc


def prepare(ent_embeddings, ent_type_ids, ent_mask):
    """Host-side compaction. Returns (embT_v, labc_v, pen_v, labs, n_v, n_pad)
    or None when no token is valid."""
    emb = np.ascontiguousarray(np.asarray(ent_embeddings, dtype=np.float32)).reshape(-1, H_DIM)
    labels = np.asarray(ent_type_ids).reshape(-1).astype(np.int64)
    mask = np.asarray(ent_mask).reshape(-1).astype(np.int64)

    valid = (mask == 1) & (labels >= 0)
    vidx = np.nonzero(valid)[0]
    n_v = len(vidx)
    if n_v == 0:
        return None

    n_pad = max(512, -(-n_v // 512) * 512)
    embT_v = np.zeros((H_DIM, n_pad), dtype=np.float32)
    embT_v[:, :n_v] = emb[vidx].T
    labc_v = np.full(n_pad, -1.0, dtype=np.float32)
    labc_v[:n_v] = labels[vidx].astype(np.float32)
    pen_v = np.full(n_pad, BIG, dtype=np.float32)
    pen_v[:n_v] = 0.0
    return embT_v, labc_v, pen_v, labels[vidx], n_v, n_pad


def make_in_maps(embT_v, labc_v, pen_v, n_pad, W_mu, b_mu, W_sigma, b_sigma):
    n_anchor = n_pad // 8
    n_itiles = (n_anchor + 127) // 128
    embP = _bf16(embT_v).reshape(6, 128, n_pad)
    # pack both weights: [128, 2, 6, 128]
    wP = np.ascontiguousarray(
        np.stack([_bf16(W_mu), _bf16(W_sigma)])      # [2, 768, 128]
        .reshape(2, 6, 128, 128).transpose(2, 0, 1, 3))
    labc_b = _bf16(labc_v)
    pen_b = _bf16(pen_v)
    in_maps = []
    for c in range(8):
        r = c * n_anchor
        labc_r = np.roll(labc_b, -r)
        labr = np.full(n_itiles * 128, -2.0, dtype=np.float32)
        labr[:n_anchor] = labc_r[:n_anchor].astype(np.float32)
        small = np.empty((128, 3 + n_itiles), dtype=np.float32)
        small[:, 0] = b_mu
        small[:, 1] = b_sigma
        small[:, 2] = -2.0 * b_mu
        small[:, 3:] = labr.reshape(n_itiles, 128).T
        im = {
            "wA": wP.reshape(128, 2 * 6 * 128),
            "small": small,
            "labc": labc_r,
            "pen": np.roll(pen_b, -r),
        }
        embR = np.roll(embP, -r, axis=2)
        for kk in range(6):
            im[f"emb{kk}"] = np.ascontiguousarray(embR[kk])
        in_maps.append(im)
    return in_maps


def kernel(ent_embeddings, ent_type_ids, ent_mask, W_mu, b_mu, W_sigma, b_sigma):
    from concourse.bass_utils import run_bass_kernel_spmd

    prep = prepare(ent_embeddings, ent_type_ids, ent_mask)
    if prep is None:
        return np.float32(0.0)
    embT_v, labc_v, pen_v, labs, n_v, n_pad = prep
    n_anchor = n_pad // 8
    n_itiles = (n_anchor + 127) // 128

    W_mu = np.ascontiguousarray(np.asarray(W_mu, dtype=np.float32))
    W_sigma = np.ascontiguousarray(np.asarray(W_sigma, dtype=np.float32))
    b_mu = np.ascontiguousarray(np.asarray(b_mu, dtype=np.float32))
    b_sigma = np.ascontiguousarray(np.asarray(b_sigma, dtype=np.float32))

    if n_pad not in _cache:
        _cache[n_pad] = _build(n_pad)
    nc = _cache[n_pad]

    in_maps = make_in_maps(embT_v, labc_v, pen_v, n_pad, W_mu, b_mu, W_sigma, b_sigma)
    res = run_bass_kernel_spmd(nc, in_maps, list(range(8)))

    num = np.empty(n_pad, dtype=np.float32)
    den = np.empty(n_pad, dtype=np.float32)
    for c in range(8):
        nd = res.results[c]["out"]          # [128, n_itiles*2]
        for t in range(n_itiles):
            m = min(128, n_anchor - t * 128)
            a = np.arange(m) + t * 128
            rows = (a + c * n_anchor) % n_pad
            num[rows] = nd[:m, 2 * t]
            den[rows] = nd[:m, 2 * t + 1]

    # host-side epilogue on the n_v real rows
    hist = np.bincount(labs, minlength=int(labs.max()) + 1)
    cnt = (hist[labs] - 1).astype(np.float64)
    sel = cnt > 0
    n_sel = max(sel.sum(), 1)
    num_v = num[:n_v].astype(np.float64)
    den_v = den[:n_v].astype(np.float64)
    safe_num = np.where(sel, num_v, 1.0)
    safe_den = np.where(sel, den_v, 1.0)
    safe_cnt = np.where(sel, cnt, 1.0)
    lf = (np.log(safe_den) - np.log(safe_num)) / np.log(2.0) + np.log2(safe_cnt)
    total = np.sum(np.where(sel, lf, 0.0)) / n_sel
    return np.float32(total)


# revision 12
# speedup vs baseline: 1.1335x; 1.1335x over previous
"""Trainium2 Bass kernel for nn_ContrastiveLearning (NT-Xent over pairwise
symmetrized-KL of diagonal Gaussians).

Math (equivalent reformulation of the reference):
  loss[i,j] = -0.25*(A[i,j] + A[j,i] + md[i,j] + md[j,i] - 2D)   (ld terms cancel)
  A[i,j]+A[j,i] + md[i,j]+md[j,i] - (q_i + q_j)
      = <u_i,s_j> + <s_i,u_j> - 2<w_i,m_j> - 2<m_i,w_j>  =: total[i,j]
  with u=1/sigma, s=sigma+mu^2, w=mu/sigma, m=mu, q_x=<mu^2,1/sigma>_x.
  Row-constant factors cancel in lf_i = -log2(num_i)+log2(den_i)+log2(cnt_i),
  so the device computes E[i,j] = exp(-0.25*(total[i,j] + q_j + pen_j) + C)
  where pen_j = BIG kills padding columns, the diagonal is killed by adding
  BIG*I to total via an extra matmul, and C keeps fp32 exp in range.
  num_i = sum_j E*[lab_i==lab_j],  den_i = sum_j E.
  cnt, log2 and the final masked mean are O(N) host work.

Sharding: anchor rows are data-parallel over 8 cores. Invalid tokens
(mask==0) are compacted out on the host, the remainder padded to a multiple
of 512. Each core receives the token axis rotated by core*N/8 so its anchors
are always tokens [0, n_anchor) — a single SPMD program with static
addressing.

Perf notes (learned from traces):
  - no gpsimd (slow, and it throttles other engines' SBUF access)
  - all matmul operands bf16 (2.4GHz PE streaming; errors average out over
    ~1M pairs, final rel err ~1e-5 vs 2e-2 budget)
  - host pre-transposes/pre-packs all arrays per-partition-contiguous and
    pre-casts to bf16, so every DMA moves multi-KB descriptors
  - phase-2 j-tiles are emitted one group behind phase-1 so the tensor queue
    always has independent work while a group's epilogue completes
  - vector reciprocal_approx_fast (18 bits) instead of reciprocal (5x)
  - single Exp activation table load (no Ln anywhere)
"""
import numpy as np

EMBED_DIM = 128
H_DIM = 768
C_SHIFT = 40.0           # global exponent shift, cancels between num and den
BIG = 1024.0 * 1024.0    # diag/pad kill: exp(-0.25*BIG) == 0 in fp32
SQRT_BIG = 1024.0

_cache = {}


def _bf16(x):
    import ml_dtypes
    return np.asarray(x, dtype=np.float32).astype(ml_dtypes.bfloat16)


def _fp8(x):
    import ml_dtypes
    return np.asarray(x, dtype=np.float32).astype(ml_dtypes.float8_e4m3)


def _build(n_pad):
    import concourse.bass as bass
    import concourse.tile as tile
    from concourse import bacc, mybir, masks

    f32 = mybir.dt.float32
    bf16 = mybir.dt.bfloat16
    fp8 = mybir.dt.float8e4
    DR = mybir.MatmulPerfMode.DoubleRow
    AF = mybir.ActivationFunctionType
    ALU = mybir.AluOpType
    AX = mybir.AxisListType

    n_groups = n_pad // 512          # 512-token groups == J-tiles
    n_anchor = n_pad // 8            # anchors per core (multiple of 64)
    n_itiles = (n_anchor + 127) // 128
    assert n_anchor <= 512

    nc = bacc.Bacc(None, target_bir_lowering=False, debug=False)
    # emb pair-slab pr: [128, 2*n_pad] fp8,
    # emb<pr>[p, kk*n_pad+t] = emb[token t, feature (2*pr+kk)*128+p]
    emb_ds = [nc.declare_dram_parameter(f"emb{pr}", [128, 2 * n_pad], fp8, isOutput=False)
              for pr in range(3)]
    # wA: [128, 2*3*2*128] fp8: wA[p, ((w*3+pr)*2+kk)*128+c] = W[(2*pr+kk)*128+p, c]
    wA_d = nc.declare_dram_parameter("wA", [128, 2 * 6 * 128], fp8, isOutput=False)
    # small: [128, 3+n_itiles] f32: [bmu, bsig, -2*bmu, labr tiles...]
    small_d = nc.declare_dram_parameter("small", [128, 3 + n_itiles], f32, isOutput=False)
    labc_d = nc.declare_dram_parameter("labc", [n_pad], bf16, isOutput=False)
    pen_d = nc.declare_dram_parameter("pen", [n_pad], bf16, isOutput=False)
    out_d = nc.declare_dram_parameter("out", [128, n_itiles * 2], f32, isOutput=True)

    with tile.TileContext(nc) as tc:
        with tc.tile_pool(name="const", bufs=1) as const, \
             tc.tile_pool(name="feat", bufs=1) as feat, \
             tc.tile_pool(name="work", bufs=2) as work, \
             tc.tile_pool(name="acc", bufs=1) as accp, \
             tc.tile_pool(name="psB", bufs=2, space="PSUM") as psB, \
             tc.tile_pool(name="psC", bufs=2, space="PSUM") as psC:

            # ---------------- bulk loads (small first, emb slabs parallel) --
            small_t = const.tile([128, 3 + n_itiles], f32)
            nc.sync.dma_start(small_t[:], small_d[:])

            w_all = const.tile([128, 2, 3, 2, 128], fp8)
            nc.sync.dma_start(w_all[:], wA_d[:])
            emb_ks = [const.tile([128, 2, n_pad], fp8, tag=f"emb{pr}", name=f"emb{pr}")
                      for pr in range(3)]
            h_ks = [const.tile([128, 2, n_pad], fp8, tag=f"h{pr}", name=f"h{pr}")
                    for pr in range(3)]

            def emb_dma(pr, g):
                for kk in range(2):
                    nc.sync.dma_start(
                        emb_ks[pr][:, kk, g * 512:(g + 1) * 512],
                        emb_ds[pr][:, kk * n_pad + g * 512:kk * n_pad + (g + 1) * 512])

            for pr in range(3):
                emb_dma(pr, 0)
            # qp row 1 = pen (row 0 = q, written per group)
            qp = feat.tile([2, n_pad], bf16)
            nc.sync.dma_start(qp[1:2, :], pen_d.rearrange("(o n) -> o n", o=1))
            labcb_t = const.tile([128, n_pad], bf16)
            nc.sync.dma_start(labcb_t[:], labc_d[:].partition_broadcast(128))
            for g in range(1, n_groups):
                for pr in range(3):
                    emb_dma(pr, g)

            # ---------------- small consts ----------------
            identity0 = const.tile([128, 128], f32)
            masks.make_identity(nc, identity0[:])
            eqd_b = const.tile([128, 128], bf16)
            nc.vector.tensor_scalar(eqd_b[:], identity0[:], SQRT_BIG, None, ALU.mult)
            onescol_b = const.tile([128, 1], bf16)
            nc.vector.memset(onescol_b[:], 1.0)
            ones2_b = const.tile([2, 128], bf16)
            nc.vector.memset(ones2_b[:], 1.0)
            cbias_t = const.tile([128, 1], f32)
            nc.vector.memset(cbias_t[:], C_SHIFT)

            # ---------------- persistent feature tensors (bf16) -----------
            u_f = feat.tile([128, n_pad], bf16)      # 1/sigma          (G_col)
            s_f = feat.tile([128, n_pad], bf16)      # sigma + mu^2     (G_col)
            m2_f = feat.tile([128, n_pad], bf16)     # -2*mu            (G_col)
            w2_f = feat.tile([128, n_pad], bf16)     # -2*mu/sigma      (G_col)
            mu_a = feat.tile([128, n_anchor], bf16)  # mu       (G_row anchors)
            w_a = feat.tile([128, n_anchor], bf16)   # mu/sigma (G_row anchors)

            num_sl = accp.tile([128, n_itiles * n_groups], f32)
            den_sl = accp.tile([128, n_itiles * n_groups], f32)

            def phase2(t, jt):
                m = min(128, n_anchor - t * 128)
                isl = slice(t * 128, t * 128 + m)
                jsl = slice(jt * 512, (jt + 1) * 512)
                doff = t * 128                   # diagonal lives in j-tile 0
                idx = t * n_groups + jt
                ps2 = psC.tile([128, 512], f32, tag="p2")
                nc.tensor.matmul(ps2[0:m, :], s_f[:, isl], u_f[:, jsl],
                                 start=True, stop=False)
                nc.tensor.matmul(ps2[0:m, :], u_f[:, isl], s_f[:, jsl],
                                 start=False, stop=False)
                nc.tensor.matmul(ps2[0:m, :], w_a[:, isl], m2_f[:, jsl],
                                 start=False, stop=False)
                nc.tensor.matmul(ps2[0:m, :], mu_a[:, isl], w2_f[:, jsl],
                                 start=False, stop=False)
                last = (jt != 0)
                nc.tensor.matmul(ps2[0:m, :], ones2_b[:, 0:m], qp[:, jsl],
                                 start=False, stop=last)
                if jt == 0:
                    nc.tensor.matmul(ps2[0:m, doff:doff + 128],
                                     eqd_b[:, 0:m], eqd_b[:],
                                     start=False, stop=True)
                e_t = work.tile([128, 512], bf16, tag="E")
                nc.scalar.activation(e_t[0:m, :], ps2[0:m, :], AF.Exp,
                                     scale=-0.25, bias=cbias_t[0:m, :],
                                     accum_out=den_sl[0:m, idx:idx + 1])
                labq = work.tile([128, 512], bf16, tag="labq")
                nc.vector.tensor_scalar(labq[0:m, :], labcb_t[0:m, jsl],
                                        small_t[0:m, 3 + t:4 + t], None, ALU.is_equal)
                msk = work.tile([128, 512], bf16, tag="msk")
                nc.vector.scalar_tensor_tensor(msk[0:m, :], e_t[0:m, :], 1.0,
                                               labq[0:m, :], ALU.mult, ALU.mult,
                                               accum_out=num_sl[0:m, idx:idx + 1])

            # ---------------- phase 1 (+ phase 2 one group behind) --------
            for g in range(n_groups):
                gs = slice(g * 512, (g + 1) * 512)
                for pr in range(3):
                    if pr % 2 == 0:
                        nc.scalar.activation(h_ks[pr][:, :, gs], emb_ks[pr][:, :, gs], AF.Relu)
                    else:
                        nc.vector.tensor_scalar_max(h_ks[pr][:, :, gs], emb_ks[pr][:, :, gs], 0.0)
                ps_mu = psB.tile([128, 512], f32, tag="mu")
                ps_z = psB.tile([128, 512], f32, tag="z")
                for pr in range(3):
                    nc.tensor.matmul(ps_mu[:], w_all[:, 0, pr, :, :],
                                     h_ks[pr][:, :, gs], perf_mode=DR,
                                     start=(pr == 0), stop=(pr == 2))
                for pr in range(3):
                    nc.tensor.matmul(ps_z[:], w_all[:, 1, pr, :, :],
                                     h_ks[pr][:, :, gs], perf_mode=DR,
                                     start=(pr == 0), stop=(pr == 2))

                # mu-path epilogue
                psq = work.tile([128, 512], f32, tag="psq")
                nc.scalar.activation(psq[:], ps_mu[:], AF.Square, bias=small_t[:, 0:1])
                nc.vector.tensor_scalar(m2_f[:, gs], ps_mu[:], small_t[:, 0:1],
                                        -2.0, ALU.add, ALU.mult)
                # sigma-path epilogue: sigma = exp(min(z,0)) + 1e-14 + relu(z)
                zm = work.tile([128, 512], f32, tag="zm")
                nc.vector.tensor_scalar(zm[:], ps_z[:], small_t[:, 1:2], 0.0,
                                        ALU.add, ALU.min)
                zp = work.tile([128, 512], f32, tag="zp")
                nc.scalar.activation(zp[:], ps_z[:], AF.Relu, bias=small_t[:, 1:2])
                e1 = work.tile([128, 512], f32, tag="e1")
                nc.scalar.activation(e1[:], zm[:], AF.Exp)
                sig_g = work.tile([128, 512], f32, tag="sig_g")
                nc.vector.scalar_tensor_tensor(sig_g[:], e1[:], 1e-14, zp[:],
                                               ALU.add, ALU.add)
                u_s = work.tile([128, 512], f32, tag="u_s")
                nc.vector.reciprocal_approx_fast(u_s[:], sig_g[:])
                nc.scalar.copy(u_f[:, gs], u_s[:])
                nc.vector.tensor_add(s_f[:, gs], psq[:], sig_g[:])
                nc.vector.tensor_mul(w2_f[:, gs], m2_f[:, gs], u_f[:, gs])
                pu_g = work.tile([128, 512], bf16, tag="pu_g")
                nc.vector.tensor_mul(pu_g[:], psq[:], u_s[:])
                if g == 0:
                    nc.vector.tensor_scalar_mul(mu_a[:], m2_f[:, 0:n_anchor], -0.5)
                    nc.vector.tensor_scalar_mul(w_a[:], w2_f[:, 0:n_anchor], -0.5)
                # q row: ones^T @ pu
                ps_q = psC.tile([1, 512], f32, tag="q")
                nc.tensor.matmul(ps_q[:], onescol_b[:], pu_g[:],
                                 start=True, stop=True)
                nc.scalar.copy(qp[0:1, gs], ps_q[:])

                # phase-2 column tiles, one group behind
                if g >= 1:
                    for t in range(n_itiles):
                        phase2(t, g - 1)
            for t in range(n_itiles):
                phase2(t, n_groups - 1)

            # ---------------- final row reductions + single store ---------
            nd = accp.tile([128, n_itiles * 2], f32)
            for t in range(n_itiles):
                sl = slice(t * n_groups, (t + 1) * n_groups)
                nc.vector.tensor_reduce(nd[:, 2 * t:2 * t + 1], num_sl[:, sl],
                                        AX.X, ALU.add)
                nc.vector.tensor_reduce(nd[:, 2 * t + 1:2 * t + 2], den_sl[:, sl],
                                        AX.X, ALU.add)
            nc.sync.dma_start(out_d[:], nd[:])

    nc.compile()
    return nc


def prepare(ent_embeddings, ent_type_ids, ent_mask):
    """Host-side compaction. Returns (embT_v, labc_v, pen_v, labs, n_v, n_pad)
    or None when no token is valid."""
    emb = np.ascontiguousarray(np.asarray(ent_embeddings, dtype=np.float32)).reshape(-1, H_DIM)
    labels = np.asarray(ent_type_ids).reshape(-1).astype(np.int64)
    mask = np.asarray(ent_mask).reshape(-1).astype(np.int64)

    valid = (mask == 1) & (labels >= 0)
    vidx = np.nonzero(valid)[0]
    n_v = len(vidx)
    if n_v == 0:
        return None

    n_pad = max(512, -(-n_v // 512) * 512)
    embT_v = np.zeros((H_DIM, n_pad), dtype=np.float32)
    embT_v[:, :n_v] = emb[vidx].T
    labc_v = np.full(n_pad, -1.0, dtype=np.float32)
    labc_v[:n_v] = labels[vidx].astype(np.float32)
    pen_v = np.full(n_pad, BIG, dtype=np.float32)
    pen_v[:n_v] = 0.0
    return embT_v, labc_v, pen_v, labels[vidx], n_v, n_pad


def make_in_maps(embT_v, labc_v, pen_v, n_pad, W_mu, b_mu, W_sigma, b_sigma):
    n_anchor = n_pad // 8
    n_itiles = (n_anchor + 127) // 128
    embP = _fp8(embT_v).reshape(6, 128, n_pad)
    # pack both weights: [128, 2(w), 3(pr), 2(kk), 128]
    wP = np.ascontiguousarray(
        np.stack([_fp8(W_mu), _fp8(W_sigma)])        # [2, 768, 128]
        .reshape(2, 3, 2, 128, 128).transpose(3, 0, 1, 2, 4))
    labc_b = _bf16(labc_v)
    pen_b = _bf16(pen_v)
    in_maps = []
    for c in range(8):
        r = c * n_anchor
        labc_r = np.roll(labc_b, -r)
        labr = np.full(n_itiles * 128, -2.0, dtype=np.float32)
        labr[:n_anchor] = labc_r[:n_anchor].astype(np.float32)
        small = np.empty((128, 3 + n_itiles), dtype=np.float32)
        small[:, 0] = b_mu
        small[:, 1] = b_sigma
        small[:, 2] = -2.0 * b_mu
        small[:, 3:] = labr.reshape(n_itiles, 128).T
        im = {
            "wA": wP.reshape(128, 2 * 6 * 128),
            "small": small,
            "labc": labc_r,
            "pen": np.roll(pen_b, -r),
        }
        embR = np.roll(embP, -r, axis=2)
        for pr in range(3):
            im[f"emb{pr}"] = np.ascontiguousarray(
                embR[2 * pr:2 * pr + 2].transpose(1, 0, 2)).reshape(128, 2 * n_pad)
        in_maps.append(im)
    return in_maps


def kernel(ent_embeddings, ent_type_ids, ent_mask, W_mu, b_mu, W_sigma, b_sigma):
    from concourse.bass_utils import run_bass_kernel_spmd

    prep = prepare(ent_embeddings, ent_type_ids, ent_mask)
    if prep is None:
        return np.float32(0.0)
    embT_v, labc_v, pen_v, labs, n_v, n_pad = prep
    n_anchor = n_pad // 8
    n_itiles = (n_anchor + 127) // 128

    W_mu = np.ascontiguousarray(np.asarray(W_mu, dtype=np.float32))
    W_sigma = np.ascontiguousarray(np.asarray(W_sigma, dtype=np.float32))
    b_mu = np.ascontiguousarray(np.asarray(b_mu, dtype=np.float32))
    b_sigma = np.ascontiguousarray(np.asarray(b_sigma, dtype=np.float32))

    if n_pad not in _cache:
        _cache[n_pad] = _build(n_pad)
    nc = _cache[n_pad]

    in_maps = make_in_maps(embT_v, labc_v, pen_v, n_pad, W_mu, b_mu, W_sigma, b_sigma)
    res = run_bass_kernel_spmd(nc, in_maps, list(range(8)))

    num = np.empty(n_pad, dtype=np.float32)
    den = np.empty(n_pad, dtype=np.float32)
    for c in range(8):
        nd = res.results[c]["out"]          # [128, n_itiles*2]
        for t in range(n_itiles):
            m = min(128, n_anchor - t * 128)
            a = np.arange(m) + t * 128
            rows = (a + c * n_anchor) % n_pad
            num[rows] = nd[:m, 2 * t]
            den[rows] = nd[:m, 2 * t + 1]

    # host-side epilogue on the n_v real rows
    hist = np.bincount(labs, minlength=int(labs.max()) + 1)
    cnt = (hist[labs] - 1).astype(np.float64)
    sel = cnt > 0
    n_sel = max(sel.sum(), 1)
    num_v = num[:n_v].astype(np.float64)
    den_v = den[:n_v].astype(np.float64)
    safe_num = np.where(sel, num_v, 1.0)
    safe_den = np.where(sel, den_v, 1.0)
    safe_cnt = np.where(sel, cnt, 1.0)
    lf = (np.log(safe_den) - np.log(safe_num)) / np.log(2.0) + np.log2(safe_cnt)
    total = np.sum(np.where(sel, lf, 0.0)) / n_sel
    return np.float32(total)


# revision 13
# speedup vs baseline: 1.1508x; 1.0153x over previous
"""Trainium2 Bass kernel for nn_ContrastiveLearning (NT-Xent over pairwise
symmetrized-KL of diagonal Gaussians).

Math (equivalent reformulation of the reference):
  loss[i,j] = -0.25*(A[i,j] + A[j,i] + md[i,j] + md[j,i] - 2D)   (ld terms cancel)
  A[i,j]+A[j,i] + md[i,j]+md[j,i] - (q_i + q_j)
      = <u_i,s_j> + <s_i,u_j> - 2<w_i,m_j> - 2<m_i,w_j>  =: total[i,j]
  with u=1/sigma, s=sigma+mu^2, w=mu/sigma, m=mu, q_x=<mu^2,1/sigma>_x.
  Row-constant factors cancel in lf_i = -log2(num_i)+log2(den_i)+log2(cnt_i),
  so the device computes E[i,j] = exp(-0.25*(total[i,j] + q_j + pen_j) + C)
  where pen_j = BIG kills padding columns, the diagonal is killed by adding
  BIG*I to total via an extra matmul, and C keeps fp32 exp in range.
  num_i = sum_j E*[lab_i==lab_j],  den_i = sum_j E.
  cnt, log2 and the final masked mean are O(N) host work.

Sharding: anchor rows are data-parallel over 8 cores. Invalid tokens
(mask==0) are compacted out on the host, the remainder padded to a multiple
of 512. Each core receives the token axis rotated by core*N/8 so its anchors
are always tokens [0, n_anchor) — a single SPMD program with static
addressing.

Perf notes (learned from traces):
  - no gpsimd (slow, and it throttles other engines' SBUF access)
  - all matmul operands bf16 (2.4GHz PE streaming; errors average out over
    ~1M pairs, final rel err ~1e-5 vs 2e-2 budget)
  - host pre-transposes/pre-packs all arrays per-partition-contiguous and
    pre-casts to bf16, so every DMA moves multi-KB descriptors
  - phase-2 j-tiles are emitted one group behind phase-1 so the tensor queue
    always has independent work while a group's epilogue completes
  - vector reciprocal_approx_fast (18 bits) instead of reciprocal (5x)
  - single Exp activation table load (no Ln anywhere)
"""
import numpy as np

EMBED_DIM = 128
H_DIM = 768
C_SHIFT = 40.0           # global exponent shift, cancels between num and den
BIG = 1024.0 * 1024.0    # diag/pad kill: exp(-0.25*BIG) == 0 in fp32
SQRT_BIG = 1024.0

_cache = {}


def _bf16(x):
    import ml_dtypes
    return np.asarray(x, dtype=np.float32).astype(ml_dtypes.bfloat16)


def _fp8(x):
    import ml_dtypes
    return np.asarray(x, dtype=np.float32).astype(ml_dtypes.float8_e4m3)


def _build(n_pad):
    import concourse.bass as bass
    import concourse.tile as tile
    from concourse import bacc, mybir, masks

    f32 = mybir.dt.float32
    bf16 = mybir.dt.bfloat16
    fp8 = mybir.dt.float8e4
    DR = mybir.MatmulPerfMode.DoubleRow
    AF = mybir.ActivationFunctionType
    ALU = mybir.AluOpType
    AX = mybir.AxisListType

    n_groups = n_pad // 512          # 512-token groups == J-tiles
    n_anchor = n_pad // 8            # anchors per core (multiple of 64)
    n_itiles = (n_anchor + 127) // 128
    assert n_anchor <= 512

    nc = bacc.Bacc(None, target_bir_lowering=False, debug=False)
    # emb pair-slab pr, group-major: emb<pr>[p, ((g*2)+kk)*512+t'] =
    #   emb[token g*512+t', feature (2*pr+kk)*128+p]
    emb_ds = [nc.declare_dram_parameter(f"emb{pr}", [128, 2 * n_pad], fp8, isOutput=False)
              for pr in range(3)]
    # wA: [128, 2*3*2*128] fp8: wA[p, ((w*3+pr)*2+kk)*128+c] = W[(2*pr+kk)*128+p, c]
    wA_d = nc.declare_dram_parameter("wA", [128, 2 * 6 * 128], fp8, isOutput=False)
    # small: [128, 3+n_itiles] f32: [bmu, bsig, -2*bmu, labr tiles...]
    small_d = nc.declare_dram_parameter("small", [128, 3 + n_itiles], f32, isOutput=False)
    labc_d = nc.declare_dram_parameter("labc", [n_pad], bf16, isOutput=False)
    pen_d = nc.declare_dram_parameter("pen", [n_pad], bf16, isOutput=False)
    out_d = nc.declare_dram_parameter("out", [128, n_itiles * 2], f32, isOutput=True)

    with tile.TileContext(nc) as tc:
        with tc.tile_pool(name="const", bufs=1) as const, \
             tc.tile_pool(name="feat", bufs=1) as feat, \
             tc.tile_pool(name="work", bufs=2) as work, \
             tc.tile_pool(name="acc", bufs=1) as accp, \
             tc.tile_pool(name="psB", bufs=2, space="PSUM") as psB, \
             tc.tile_pool(name="psC", bufs=2, space="PSUM") as psC:

            # ---------------- bulk loads (g0 emb first, then the rest) -----
            emb_ks = [const.tile([128, n_groups, 2, 512], fp8, tag=f"emb{pr}", name=f"emb{pr}")
                      for pr in range(3)]
            h_ks = [const.tile([128, n_groups, 2, 512], fp8, tag=f"h{pr}", name=f"h{pr}")
                    for pr in range(3)]

            def emb_dma(pr, g):
                nc.sync.dma_start(
                    emb_ks[pr][:, g, :, :],
                    emb_ds[pr][:, g * 1024:(g + 1) * 1024])

            for pr in range(3):
                emb_dma(pr, 0)
            small_t = const.tile([128, 3 + n_itiles], f32)
            nc.sync.dma_start(small_t[:], small_d[:])

            w_all = const.tile([128, 2, 3, 2, 128], fp8)
            nc.sync.dma_start(w_all[:], wA_d[:])
            # qp row 1 = pen (row 0 = q, written per group)
            qp = feat.tile([2, n_pad], bf16)
            nc.sync.dma_start(qp[1:2, :], pen_d.rearrange("(o n) -> o n", o=1))
            labcb_t = const.tile([128, n_pad], bf16)
            nc.sync.dma_start(labcb_t[:], labc_d[:].partition_broadcast(128))
            for g in range(1, n_groups):
                for pr in range(3):
                    emb_dma(pr, g)

            # ---------------- small consts ----------------
            identity0 = const.tile([128, 128], f32)
            masks.make_identity(nc, identity0[:])
            eqd_b = const.tile([128, 128], bf16)
            nc.vector.tensor_scalar(eqd_b[:], identity0[:], SQRT_BIG, None, ALU.mult)
            onescol_b = const.tile([128, 1], bf16)
            nc.vector.memset(onescol_b[:], 1.0)
            ones2_b = const.tile([2, 128], bf16)
            nc.vector.memset(ones2_b[:], 1.0)
            cbias_t = const.tile([128, 1], f32)
            nc.vector.memset(cbias_t[:], C_SHIFT)
            # preload the Exp activation table while engines are idle
            dum = const.tile([1, 1], f32)
            nc.scalar.activation(dum[:], cbias_t[0:1, :], AF.Exp)

            # ---------------- persistent feature tensors (bf16) -----------
            u_f = feat.tile([128, n_pad], bf16)      # 1/sigma          (G_col)
            s_f = feat.tile([128, n_pad], bf16)      # sigma + mu^2     (G_col)
            m2_f = feat.tile([128, n_pad], bf16)     # -2*mu            (G_col)
            w2_f = feat.tile([128, n_pad], bf16)     # -2*mu/sigma      (G_col)
            mu_a = feat.tile([128, n_anchor], bf16)  # mu       (G_row anchors)
            w_a = feat.tile([128, n_anchor], bf16)   # mu/sigma (G_row anchors)

            num_sl = accp.tile([128, n_itiles * n_groups], f32)
            den_sl = accp.tile([128, n_itiles * n_groups], f32)

            def phase2(t, jt):
                m = min(128, n_anchor - t * 128)
                isl = slice(t * 128, t * 128 + m)
                jsl = slice(jt * 512, (jt + 1) * 512)
                doff = t * 128                   # diagonal lives in j-tile 0
                idx = t * n_groups + jt
                ps2 = psC.tile([128, 512], f32, tag="p2")
                nc.tensor.matmul(ps2[0:m, :], s_f[:, isl], u_f[:, jsl],
                                 start=True, stop=False)
                nc.tensor.matmul(ps2[0:m, :], u_f[:, isl], s_f[:, jsl],
                                 start=False, stop=False)
                nc.tensor.matmul(ps2[0:m, :], w_a[:, isl], m2_f[:, jsl],
                                 start=False, stop=False)
                nc.tensor.matmul(ps2[0:m, :], mu_a[:, isl], w2_f[:, jsl],
                                 start=False, stop=False)
                last = (jt != 0)
                nc.tensor.matmul(ps2[0:m, :], ones2_b[:, 0:m], qp[:, jsl],
                                 start=False, stop=last)
                if jt == 0:
                    nc.tensor.matmul(ps2[0:m, doff:doff + 128],
                                     eqd_b[:, 0:m], eqd_b[:],
                                     start=False, stop=True)
                e_t = work.tile([128, 512], bf16, tag="E")
                nc.scalar.activation(e_t[0:m, :], ps2[0:m, :], AF.Exp,
                                     scale=-0.25, bias=cbias_t[0:m, :],
                                     accum_out=den_sl[0:m, idx:idx + 1])
                labq = work.tile([128, 512], bf16, tag="labq")
                nc.vector.tensor_scalar(labq[0:m, :], labcb_t[0:m, jsl],
                                        small_t[0:m, 3 + t:4 + t], None, ALU.is_equal)
                msk = work.tile([128, 512], bf16, tag="msk")
                nc.vector.scalar_tensor_tensor(msk[0:m, :], e_t[0:m, :], 1.0,
                                               labq[0:m, :], ALU.mult, ALU.mult,
                                               accum_out=num_sl[0:m, idx:idx + 1])

            # ---------------- phase 1 (+ phase 2 one group behind) --------
            for g in range(n_groups):
                gs = slice(g * 512, (g + 1) * 512)
                for pr in range(3):
                    if pr % 2 == 0:
                        nc.scalar.activation(h_ks[pr][:, g, :, :], emb_ks[pr][:, g, :, :], AF.Relu)
                    else:
                        nc.vector.tensor_scalar_max(h_ks[pr][:, g, :, :], emb_ks[pr][:, g, :, :], 0.0)
                ps_mu = psB.tile([128, 512], f32, tag="mu")
                ps_z = psB.tile([128, 512], f32, tag="z")
                for pr in range(3):
                    nc.tensor.matmul(ps_mu[:], w_all[:, 0, pr, :, :],
                                     h_ks[pr][:, g, :, :], perf_mode=DR,
                                     start=(pr == 0), stop=(pr == 2))
                for pr in range(3):
                    nc.tensor.matmul(ps_z[:], w_all[:, 1, pr, :, :],
                                     h_ks[pr][:, g, :, :], perf_mode=DR,
                                     start=(pr == 0), stop=(pr == 2))

                # mu-path epilogue
                psq = work.tile([128, 512], f32, tag="psq")
                nc.scalar.activation(psq[:], ps_mu[:], AF.Square, bias=small_t[:, 0:1])
                nc.vector.tensor_scalar(m2_f[:, gs], ps_mu[:], small_t[:, 0:1],
                                        -2.0, ALU.add, ALU.mult)
                # sigma-path epilogue: sigma = exp(min(z,0)) + 1e-14 + relu(z)
                zm = work.tile([128, 512], f32, tag="zm")
                nc.vector.tensor_scalar(zm[:], ps_z[:], small_t[:, 1:2], 0.0,
                                        ALU.add, ALU.min)
                zp = work.tile([128, 512], f32, tag="zp")
                nc.scalar.activation(zp[:], ps_z[:], AF.Relu, bias=small_t[:, 1:2])
                e1 = work.tile([128, 512], f32, tag="e1")
                nc.scalar.activation(e1[:], zm[:], AF.Exp)
                sig_g = work.tile([128, 512], f32, tag="sig_g")
                nc.vector.scalar_tensor_tensor(sig_g[:], e1[:], 1e-14, zp[:],
                                               ALU.add, ALU.add)
                u_s = work.tile([128, 512], f32, tag="u_s")
                nc.vector.reciprocal_approx_fast(u_s[:], sig_g[:])
                nc.scalar.copy(u_f[:, gs], u_s[:])
                nc.vector.tensor_add(s_f[:, gs], psq[:], sig_g[:])
                nc.vector.tensor_mul(w2_f[:, gs], m2_f[:, gs], u_f[:, gs])
                pu_g = work.tile([128, 512], bf16, tag="pu_g")
                nc.vector.tensor_mul(pu_g[:], psq[:], u_s[:])
                if g == 0:
                    nc.vector.tensor_scalar_mul(mu_a[:], m2_f[:, 0:n_anchor], -0.5)
                    nc.vector.tensor_scalar_mul(w_a[:], w2_f[:, 0:n_anchor], -0.5)
                # q row: ones^T @ pu
                ps_q = psC.tile([1, 512], f32, tag="q")
                nc.tensor.matmul(ps_q[:], onescol_b[:], pu_g[:],
                                 start=True, stop=True)
                nc.scalar.copy(qp[0:1, gs], ps_q[:])

                # phase-2 column tiles, one group behind
                if g >= 1:
                    for t in range(n_itiles):
                        phase2(t, g - 1)
            for t in range(n_itiles):
                phase2(t, n_groups - 1)

            # ---------------- final row reductions + single store ---------
            nd = accp.tile([128, n_itiles * 2], f32)
            for t in range(n_itiles):
                sl = slice(t * n_groups, (t + 1) * n_groups)
                nc.vector.tensor_reduce(nd[:, 2 * t:2 * t + 1], num_sl[:, sl],
                                        AX.X, ALU.add)
                nc.vector.tensor_reduce(nd[:, 2 * t + 1:2 * t + 2], den_sl[:, sl],
                                        AX.X, ALU.add)
            nc.sync.dma_start(out_d[:], nd[:])

    nc.compile()
    return nc


def prepare(ent_embeddings, ent_type_ids, ent_mask):
    """Host-side compaction. Returns (embT_v, labc_v, pen_v, labs, n_v, n_pad)
    or None when no token is valid."""
    emb = np.ascontiguousarray(np.asarray(ent_embeddings, dtype=np.float32)).reshape(-1, H_DIM)
    labels = np.asarray(ent_type_ids).reshape(-1).astype(np.int64)
    mask = np.asarray(ent_mask).reshape(-1).astype(np.int64)

    valid = (mask == 1) & (labels >= 0)
    vidx = np.nonzero(valid)[0]
    n_v = len(vidx)
    if n_v == 0:
        return None

    n_pad = max(512, -(-n_v // 512) * 512)
    embT_v = np.zeros((H_DIM, n_pad), dtype=np.float32)
    embT_v[:, :n_v] = emb[vidx].T
    labc_v = np.full(n_pad, -1.0, dtype=np.float32)
    labc_v[:n_v] = labels[vidx].astype(np.float32)
    pen_v = np.full(n_pad, BIG, dtype=np.float32)
    pen_v[:n_v] = 0.0
    return embT_v, labc_v, pen_v, labels[vidx], n_v, n_pad


def make_in_maps(embT_v, labc_v, pen_v, n_pad, W_mu, b_mu, W_sigma, b_sigma):
    n_anchor = n_pad // 8
    n_itiles = (n_anchor + 127) // 128
    embP = _fp8(embT_v).reshape(6, 128, n_pad)
    # pack both weights: [128, 2(w), 3(pr), 2(kk), 128]
    wP = np.ascontiguousarray(
        np.stack([_fp8(W_mu), _fp8(W_sigma)])        # [2, 768, 128]
        .reshape(2, 3, 2, 128, 128).transpose(3, 0, 1, 2, 4))
    labc_b = _bf16(labc_v)
    pen_b = _bf16(pen_v)
    in_maps = []
    for c in range(8):
        r = c * n_anchor
        labc_r = np.roll(labc_b, -r)
        labr = np.full(n_itiles * 128, -2.0, dtype=np.float32)
        labr[:n_anchor] = labc_r[:n_anchor].astype(np.float32)
        small = np.empty((128, 3 + n_itiles), dtype=np.float32)
        small[:, 0] = b_mu
        small[:, 1] = b_sigma
        small[:, 2] = -2.0 * b_mu
        small[:, 3:] = labr.reshape(n_itiles, 128).T
        im = {
            "wA": wP.reshape(128, 2 * 6 * 128),
            "small": small,
            "labc": labc_r,
            "pen": np.roll(pen_b, -r),
        }
        embR = np.roll(embP, -r, axis=2)      # [6, 128, n_pad]
        n_groups = n_pad // 512
        for pr in range(3):
            # [2, 128, n_pad] -> [128, n_groups, 2, 512]
            blk = embR[2 * pr:2 * pr + 2].reshape(2, 128, n_groups, 512)
            im[f"emb{pr}"] = np.ascontiguousarray(
                blk.transpose(1, 2, 0, 3)).reshape(128, 2 * n_pad)
        in_maps.append(im)
    return in_maps


def kernel(ent_embeddings, ent_type_ids, ent_mask, W_mu, b_mu, W_sigma, b_sigma):
    from concourse.bass_utils import run_bass_kernel_spmd

    prep = prepare(ent_embeddings, ent_type_ids, ent_mask)
    if prep is None:
        return np.float32(0.0)
    embT_v, labc_v, pen_v, labs, n_v, n_pad = prep
    n_anchor = n_pad // 8
    n_itiles = (n_anchor + 127) // 128

    W_mu = np.ascontiguousarray(np.asarray(W_mu, dtype=np.float32))
    W_sigma = np.ascontiguousarray(np.asarray(W_sigma, dtype=np.float32))
    b_mu = np.ascontiguousarray(np.asarray(b_mu, dtype=np.float32))
    b_sigma = np.ascontiguousarray(np.asarray(b_sigma, dtype=np.float32))

    if n_pad not in _cache:
        _cache[n_pad] = _build(n_pad)
    nc = _cache[n_pad]

    in_maps = make_in_maps(embT_v, labc_v, pen_v, n_pad, W_mu, b_mu, W_sigma, b_sigma)
    res = run_bass_kernel_spmd(nc, in_maps, list(range(8)))

    num = np.empty(n_pad, dtype=np.float32)
    den = np.empty(n_pad, dtype=np.float32)
    for c in range(8):
        nd = res.results[c]["out"]          # [128, n_itiles*2]
        for t in range(n_itiles):
            m = min(128, n_anchor - t * 128)
            a = np.arange(m) + t * 128
            rows = (a + c * n_anchor) % n_pad
            num[rows] = nd[:m, 2 * t]
            den[rows] = nd[:m, 2 * t + 1]

    # host-side epilogue on the n_v real rows
    hist = np.bincount(labs, minlength=int(labs.max()) + 1)
    cnt = (hist[labs] - 1).astype(np.float64)
    sel = cnt > 0
    n_sel = max(sel.sum(), 1)
    num_v = num[:n_v].astype(np.float64)
    den_v = den[:n_v].astype(np.float64)
    safe_num = np.where(sel, num_v, 1.0)
    safe_den = np.where(sel, den_v, 1.0)
    safe_cnt = np.where(sel, cnt, 1.0)
    lf = (np.log(safe_den) - np.log(safe_num)) / np.log(2.0) + np.log2(safe_cnt)
    total = np.sum(np.where(sel, lf, 0.0)) / n_sel
    return np.float32(total)
